# revision 6
# baseline (speedup 1.0000x reference)
"""BERT attention layer (B=4, S=1024, H=1024, NH=16) on 8 TRN2 NeuronCores.

Sharding: core c -> batch b = c//2, sequence-half = c%2.  Each core computes
full K/V for its batch, attention + output projection + residual layernorm
for its 512 query rows, and writes a disjoint [512, 1024] slice of the
output.  The host rolls the sequence axis by 512 for odd cores so one SPMD
program serves all 8 cores (softmax over keys is permutation invariant).

Key optimizations over the naive port:
- All 13 logical inputs are packed host-side into TWO DRAM tensors (one fp8
  bundle: x^T + the four weights; one f32 aux bundle: residual rows, biases,
  mask, gamma/beta).  Per-call dispatch cost through jax/axon scales with
  the number of bound tensors, so this cuts most of the per-exec overhead.
- QKV/output projections and the probs@V contraction run fp8e4m3 with
  DoubleRow perf mode (256-deep contraction per PE pass); attention scores
  (Q.K^T, 64-deep) stay bf16 where softmax is most error-sensitive.
  End-to-end rel err ~1.5e-3 vs the f32 reference.
- Softmax denominators ride the probs@V matmul as an augmented ones-row of
  V.  Each pair's ctx PSUM is copied to SBUF with one DVE op per head so
  the banks free immediately; one reciprocal + one broadcast DMA cover the
  pair and the normalize multiplies run on the idle Pool engine off the
  critical path.
- Input DMAs are single 3D-AP transfers spread across the SP/ACT/SWDGE
  queues; Q/K tile 0 is computed up front and the rest of Q/K/V interleaves
  into the attention pair stream just in time; layernorm affine runs on the
  Pool engine; residual+output-bias rows are pre-added during attention.
"""

import sys

for _p in ("/opt/trn_rl_repo", "/root/.axon_site/_ro/trn_rl_repo"):
    if _p not in sys.path:
        sys.path.insert(0, _p)

import numpy as np

B, S, H, NH, HS = 4, 1024, 1024, 16, 64
P = 128
QR = 512          # query rows per core
EPS = 1e-12
N_CORES = 8

# row offsets inside the f32 aux bundle
R_XRES = 0            # [512, 1024]
R_QB = 512            # [1, 1024]
R_KB = 513
R_MASK = 514
R_VB = 515
R_OB = 516
R_GAMMA = 517
R_BETA = 518
AUX_ROWS = 519

_CACHE = {}


def _build_nc():
    import concourse.mybir as mybir
    import concourse.tile as tile
    from concourse import bacc

    f32 = mybir.dt.float32
    bf16 = mybir.dt.bfloat16
    f8 = mybir.dt.float8e4
    DR = mybir.MatmulPerfMode.DoubleRow
    Alu = mybir.AluOpType
    Act = mybir.ActivationFunctionType

    nc = bacc.Bacc("TRN2", target_bir_lowering=False)

    # ---- packed per-core DRAM I/O ----
    # wx rows: [0:1024] x^T, [1024:2048] qw^T, [2048:3072] kw^T,
    #          [3072:4096] vw^T, [4096:5120] ow^T   (all [in, out] layout)
    wx_d = nc.declare_dram_parameter("wx", [5 * H, S], f8, isOutput=False)
    aux_d = nc.declare_dram_parameter("aux", [AUX_ROWS, H], f32, isOutput=False)
    out_d = nc.declare_dram_parameter("out", [QR, H], f32, isOutput=True)

    xt_d = wx_d[0:H, :]
    qwt_d = wx_d[H : 2 * H, :]
    kwt_d = wx_d[2 * H : 3 * H, :]
    vwt_d = wx_d[3 * H : 4 * H, :]
    owt_d = wx_d[4 * H : 5 * H, :]
    xres_d = aux_d[R_XRES:QR, :]

    KT8 = H // P   # 8 tiles along any 1024 dim

    with tile.TileContext(nc) as tc:
        with (
            tc.tile_pool(name="consts", bufs=1) as consts,
            tc.tile_pool(name="bigs", bufs=1) as bigs,
            tc.tile_pool(name="wbuf", bufs=4) as wbuf,
            tc.tile_pool(name="exps", bufs=3) as exps,
            tc.tile_pool(name="small", bufs=4) as small,
            tc.tile_pool(name="resid", bufs=2) as resid,
            tc.tile_pool(name="onorm", bufs=2) as onorm_pool,
            tc.tile_pool(name="norm", bufs=2) as norm_pool,
            tc.tile_pool(name="mm_ps", bufs=2, space="PSUM") as mm_ps,
            tc.tile_pool(name="sc_ps", bufs=2, space="PSUM") as sc_ps,
            tc.tile_pool(name="ctx_ps", bufs=1, space="PSUM") as ctx_ps,
            tc.tile_pool(name="dram", bufs=2, space="DRAM") as dram_pool,
        ):
            # ---- constants / small inputs (from aux rows) ----
            qkm_sb = consts.tile([P, 3, KT8], f32)
            nc.sync.dma_start(
                out=qkm_sb,
                in_=aux_d[R_QB : R_QB + 3, :].rearrange("r (io p) -> p r io", p=P),
            )
            qb_sb = qkm_sb[:, 0, :]
            kb_sb = qkm_sb[:, 1, :]
            mask_sb = qkm_sb[:, 2, :]
            vbb_sb = consts.tile([P, H], f32)
            nc.sync.dma_start(
                out=vbb_sb, in_=aux_d[R_VB : R_VB + 1, :].to_broadcast([P, H])
            )
            obb_sb = consts.tile([P, H], f32)
            nc.sync.dma_start(
                out=obb_sb, in_=aux_d[R_OB : R_OB + 1, :].to_broadcast([P, H])
            )
            gammab_sb = consts.tile([P, H], f32)
            nc.sync.dma_start(
                out=gammab_sb,
                in_=aux_d[R_GAMMA : R_GAMMA + 1, :].to_broadcast([P, H]),
            )
            betab_sb = consts.tile([P, H], f32)
            nc.sync.dma_start(
                out=betab_sb, in_=aux_d[R_BETA : R_BETA + 1, :].to_broadcast([P, H])
            )
            eps_sb = consts.tile([P, 1], f32)
            nc.vector.memset(eps_sb, EPS)
            # warm the ACT exp table during the load phase so the first real
            # exp doesn't pay the ~2.7us ACT_TABLE_LOAD
            actwarm = consts.tile([P, 1], f32)
            nc.scalar.activation(out=actwarm, in_=eps_sb, func=Act.Exp)

            # ---- x^T in SBUF: [128(i), 8(i_outer), 1024(s)] bf16 ----
            xt_sb = bigs.tile([P, KT8, S], f8)
            xt_r = xt_d.rearrange("(io p) s -> p io s", p=P)
            nc.sync.dma_start(out=xt_sb, in_=xt_r)

            # ---- persistent big tensors ----
            QT = bigs.tile([P, KT8, QR], bf16)        # Q^T  [o, q]
            KT = bigs.tile([P, KT8, S], bf16)         # K^T  [o, s]
            Vaug = bigs.tile([P, KT8, NH, HS + 1], f8)  # V   [s, h, d(+ones)]
            ctxT = bigs.tile([P, KT8, QR], f8)        # ctx^T [j, q]

            nc.vector.memset(Vaug[:, :, :, HS : HS + 1], 1.0)

            def load_wt(dram, eng):
                w = wbuf.tile([P, KT8, H], f8, tag="wt")
                w_r = dram.rearrange("(io p) o -> p io o", p=P)
                eng.dma_start(out=w, in_=w_r)
                return w

            # ---- weight loads on separate DGE queues (concurrent) ----
            qwt = load_wt(qwt_d, nc.scalar)
            kwt = load_wt(kwt_d, nc.gpsimd)
            vwt = load_wt(vwt_d, nc.sync)
            owt = load_wt(owt_d, nc.scalar)

            def emit_v(st, oh):
                ps = mm_ps.tile([P, QR], f32, tag="mm")
                for it in range(KT8 // 2):
                    nc.tensor.matmul(
                        ps,
                        xt_sb[:, 2 * it : 2 * it + 2, st * P : (st + 1) * P],
                        vwt[:, 2 * it : 2 * it + 2, oh * QR : (oh + 1) * QR],
                        start=(it == 0),
                        stop=(it == KT8 // 2 - 1),
                        perf_mode=DR,
                    )
                nc.vector.tensor_tensor(
                    out=Vaug[:, st, oh * (NH // 2) : (oh + 1) * (NH // 2), 0:HS],
                    in0=ps.rearrange("p (h d) -> p h d", d=HS),
                    in1=vbb_sb[:, oh * QR : (oh + 1) * QR].rearrange(
                        "p (h d) -> p h d", d=HS
                    ),
                    op=Alu.add,
                )

            def emit_q(ot):
                ps = mm_ps.tile([P, QR], f32, tag="mm")
                for it in range(KT8 // 2):
                    nc.tensor.matmul(
                        ps,
                        qwt[:, 2 * it : 2 * it + 2, ot * P : (ot + 1) * P],
                        xt_sb[:, 2 * it : 2 * it + 2, 0:QR],
                        start=(it == 0),
                        stop=(it == KT8 // 2 - 1),
                        perf_mode=DR,
                    )
                nc.vector.tensor_scalar_add(
                    out=QT[:, ot, :], in0=ps, scalar1=qb_sb[:, ot : ot + 1]
                )

            # prefetch residual rows and fold in the output bias while the
            # attention stage runs (Pool engine is idle then)
            xres_r = xres_d.rearrange("(st p) m -> p st m", p=P)
            xo = bigs.tile([P, QR // P, H], f32)
            for st in range(QR // P):
                xr = resid.tile([P, H], f32, tag="xr")
                nc.sync.dma_start(out=xr, in_=xres_r[:, st, :])
                nc.gpsimd.tensor_tensor(
                    out=xo[:, st, :], in0=xr, in1=obb_sb, op=Alu.add
                )

            # ---- attention ----
            def emit_kt(ot):
                for sh in range(2):
                    kps = mm_ps.tile([P, QR], f32, tag="mm", name="kps")
                    for it in range(KT8 // 2):
                        nc.tensor.matmul(
                            kps,
                            kwt[:, 2 * it : 2 * it + 2, ot * P : (ot + 1) * P],
                            xt_sb[:, 2 * it : 2 * it + 2, sh * QR : (sh + 1) * QR],
                            start=(it == 0),
                            stop=(it == KT8 // 2 - 1),
                            perf_mode=DR,
                        )
                    nc.vector.tensor_scalar_add(
                        out=KT[:, ot, sh * QR : (sh + 1) * QR],
                        in0=kps,
                        scalar1=kb_sb[:, ot : ot + 1],
                    )

            emit_q(0)
            emit_kt(0)
            emit_q(1)

            # V groups interleave into the attention stream just in time:
            # pair 0 consumes oh=0 s-tiles as its ctx matmuls need them,
            # pairs 1..3 cover the oh=1 half (needed from pair 4 on)
            v1_sched = {1: [0, 1, 2], 2: [3, 4, 5], 3: [6, 7]}

            for oo in range(NH // 2):
                heads = (2 * oo, 2 * oo + 1)  # partition offsets 0, 64

                expS = exps.tile([P, KT8, 2, QR], f8, tag="expS", name="expS")
                cpss = [
                    ctx_ps.tile([P, QR], f32, tag=f"ctx{j}", name=f"ctx{j}")
                    for j in range(2)
                ]

                def emit_scores(so):
                    sps = sc_ps.tile([P, 2 * QR], f32, tag="sc", name="sps")
                    for j in range(2):
                        po = j * HS
                        nc.tensor.matmul(
                            sps[:, j * QR : (j + 1) * QR],
                            KT[po : po + HS, oo, so * P : (so + 1) * P],
                            QT[po : po + HS, oo, :],
                            start=True,
                            stop=True,
                        )
                    nc.scalar.activation(
                        out=expS[:, so, :, :],
                        in_=sps.rearrange("p (j q) -> p j q", q=QR),
                        func=Act.Exp,
                        bias=mask_sb[:, so : so + 1],
                        scale=0.125,
                    )

                def emit_ctx_pair(t):
                    for j, h in enumerate(heads):
                        nc.tensor.matmul(
                            cpss[j][0 : HS + 1, :],
                            Vaug[:, 2 * t : 2 * t + 2, h, :],
                            expS[:, 2 * t : 2 * t + 2, j, :],
                            start=(t == 0),
                            stop=(t == KT8 // 2 - 1),
                            perf_mode=DR,
                        )

                vq = list(v1_sched.get(oo, []))
                emit_scores(0)
                emit_scores(1)
                if oo == 0:
                    emit_v(0, 0)
                    emit_v(1, 0)
                emit_ctx_pair(0)
                if oo + 1 < NH // 2:
                    emit_kt(oo + 1)
                if oo + 2 < NH // 2:
                    emit_q(oo + 2)
                for t in range(1, KT8 // 2):
                    emit_scores(2 * t)
                    emit_scores(2 * t + 1)
                    if oo == 0:
                        emit_v(2 * t, 0)
                        emit_v(2 * t + 1, 0)
                    elif vq:
                        emit_v(vq.pop(0), 1)
                    emit_ctx_pair(t)
                while vq:
                    emit_v(vq.pop(0), 1)
                # normalize by softmax denominator (row HS of cps).
                # Both heads' psums are copied into one SBUF tile with two
                # cheap DVE ops (banks free immediately); one reciprocal and
                # one broadcast DMA cover the pair, and the multiplies run
                # on the idle Pool engine off the critical path.
                cc = norm_pool.tile([HS + 1, 2, QR], f32, tag="cc")
                for j in range(2):
                    nc.vector.tensor_copy(
                        out=cc[:, j, :], in_=cpss[j][0 : HS + 1, :]
                    )
                rd = norm_pool.tile([1, 2, QR], f32, tag="rd")
                nc.vector.reciprocal(out=rd, in_=cc[HS : HS + 1, :, :])
                rdd = dram_pool.tile([1, 2 * QR], f32, tag="rdd")
                nc.gpsimd.dma_start(
                    out=rdd, in_=rd.rearrange("o j q -> o (j q)")
                )
                rdb = norm_pool.tile([P, 2, QR], f32, tag="rdb")
                nc.gpsimd.dma_start(
                    out=rdb[0:HS, :, :],
                    in_=rdd[:, :].rearrange("o (j q) -> o j q", q=QR).to_broadcast(
                        [HS, 2, QR]
                    ),
                )
                nc.gpsimd.tensor_tensor(
                    out=ctxT[0:HS, oo, :],
                    in0=cc[0:HS, 0, :],
                    in1=rdb[0:HS, 0, :],
                    op=Alu.mult,
                )
                stage = small.tile([HS, QR], f8, tag="cstage")
                nc.gpsimd.tensor_tensor(
                    out=stage, in0=cc[0:HS, 1, :], in1=rdb[0:HS, 1, :],
                    op=Alu.mult,
                )
                nc.gpsimd.dma_start(out=ctxT[HS : 2 * HS, oo, :], in_=stage)

            # ---- y = ctx @ ow^T + (ob + residual) ----
            # drain each psum half with a single add against the prefetched
            # (residual + output bias) rows, then layernorm on the result
            ysb = bigs.tile([P, QR // P, H], f32)
            for st in range(QR // P):
                for oh in range(2):
                    ps = mm_ps.tile([P, QR], f32, tag="mm")
                    for jo in range(KT8 // 2):
                        nc.tensor.matmul(
                            ps,
                            ctxT[:, 2 * jo : 2 * jo + 2, st * P : (st + 1) * P],
                            owt[:, 2 * jo : 2 * jo + 2, oh * QR : (oh + 1) * QR],
                            start=(jo == 0),
                            stop=(jo == KT8 // 2 - 1),
                            perf_mode=DR,
                        )
                    nc.vector.tensor_tensor(
                        out=ysb[:, st, oh * QR : (oh + 1) * QR],
                        in0=ps,
                        in1=xo[:, st, oh * QR : (oh + 1) * QR],
                        op=Alu.add,
                    )

                # ---- layernorm over the 1024 free dim ----
                yr = ysb[:, st, :].rearrange("p (g d) -> p g d", d=QR)
                stats = small.tile([P, 2, 6], f32, tag="stats")
                for g in range(2):
                    nc.vector.bn_stats(out=stats[:, g, :], in_=yr[:, g, :])
                mv = small.tile([P, 2], f32, tag="mv")
                nc.vector.bn_aggr(out=mv, in_=stats)
                rstd = small.tile([P, 1], f32, tag="rstd")
                nc.scalar.activation(
                    out=rstd, in_=mv[:, 1:2], func=Act.Sqrt, bias=eps_sb, scale=1.0
                )
                nc.vector.reciprocal(out=rstd, in_=rstd)
                on = onorm_pool.tile([P, H], f32, tag="on")
                nc.vector.tensor_scalar(
                    out=on,
                    in0=ysb[:, st, :],
                    scalar1=mv[:, 0:1],
                    scalar2=rstd,
                    op0=Alu.subtract,
                    op1=Alu.mult,
                )
                nc.gpsimd.tensor_tensor(out=on, in0=on, in1=gammab_sb, op=Alu.mult)
                nc.gpsimd.tensor_tensor(out=on, in0=on, in1=betab_sb, op=Alu.add)
                nc.scalar.dma_start(
                    out=out_d.rearrange("(st p) m -> p st m", p=P)[:, st, :], in_=on
                )

    nc.compile()
    return nc


def _get_nc():
    if "nc" not in _CACHE:
        _CACHE["nc"] = _build_nc()
    return _CACHE["nc"]


def _make_in_maps(inputs):
    import ml_dtypes

    f8 = ml_dtypes.float8_e4m3
    hs = np.asarray(inputs["hidden_states"], dtype=np.float32).reshape(B, S, H)
    am = np.asarray(inputs["attention_mask"], dtype=np.float32).reshape(B, S)

    # shared fp8 weight block [4096, 1024]: qw^T, kw^T, vw^T, ow^T
    wblk = np.empty((4 * H, H), dtype=f8)
    for i, nm in enumerate(("qw", "kw", "vw", "ow")):
        wblk[i * H : (i + 1) * H] = np.asarray(inputs[nm], np.float32).T.astype(f8)

    # shared aux rows (everything but xres + mask)
    aux_shared = np.zeros((AUX_ROWS - QR, H), dtype=np.float32)
    aux_shared[R_QB - QR] = np.asarray(inputs["qb"], np.float32)
    aux_shared[R_KB - QR] = np.asarray(inputs["kb"], np.float32)
    aux_shared[R_VB - QR] = np.asarray(inputs["vb"], np.float32)
    aux_shared[R_OB - QR] = np.asarray(inputs["ob"], np.float32)
    aux_shared[R_GAMMA - QR] = np.asarray(inputs["gamma"], np.float32)
    aux_shared[R_BETA - QR] = np.asarray(inputs["beta"], np.float32)

    in_maps = []
    for c in range(N_CORES):
        b, half = divmod(c, 2)
        x = hs[b]
        m = am[b]
        if half:
            x = np.roll(x, -QR, axis=0)
            m = np.roll(m, -QR)
        wx = np.empty((5 * H, S), dtype=f8)
        wx[0:H] = x.T.astype(f8)
        wx[H:] = wblk
        aux = np.empty((AUX_ROWS, H), dtype=np.float32)
        aux[0:QR] = x[:QR]
        aux[QR:] = aux_shared
        aux[R_MASK] = m
        in_maps.append({"wx": wx, "aux": aux})
    return in_maps


def _gather(results):
    out = np.empty((B, S, H), dtype=np.float32)
    for c in range(N_CORES):
        b, half = divmod(c, 2)
        out[b, half * QR : (half + 1) * QR, :] = results[c]["out"]
    return out


def run_on_hw(inputs, **kwargs):
    """Run on the 8 NeuronCores; returns (full_output, BassKernelResults)."""
    from concourse import bass_utils

    nc = _get_nc()
    in_maps = _make_in_maps(inputs)
    res = bass_utils.run_bass_kernel_spmd(
        nc, in_maps, core_ids=list(range(N_CORES)), **kwargs
    )
    return _gather(res.results), res


def kernel(**inputs) -> np.ndarray:
    out, _ = run_on_hw(inputs)
    return out


# revision 7
# speedup vs baseline: 1.0073x; 1.0073x over previous
"""BERT attention layer (B=4, S=1024, H=1024, NH=16) on 8 TRN2 NeuronCores.

v7: v5 + critical-path fixes: each attention pair's ctx PSUM is copied to
SBUF with one cheap DVE op so the PSUM banks free immediately (the next
pair's ctx no longer stalls on the softmax-denominator normalize chain);
the normalize multiply runs on the idle Pool engine from the SBUF copy and
its broadcast DMAs ride the gpsimd SWDGE queue; output DMA rides the ACT
queue (idle at the tail).
Base: packed 2-tensor input signature (v2) + fp8e4m3 DoubleRow matmuls for
the QKV/output projections and the probs@V contraction (256-deep contraction
per PE pass, 2x tensor-engine throughput).  Scores (Q.K^T) stay bf16 -- the
64-deep head contraction cannot use DoubleRow and softmax is most
error-sensitive there.

Sharding: core c -> batch b = c//2, sequence-half = c%2.  Each core computes
full K/V for its batch, attention + output projection + residual layernorm
for its 512 query rows, and writes a disjoint [512, 1024] slice of the
output.  The host rolls the sequence axis by 512 for odd cores so one SPMD
program serves all 8 cores.
"""

import sys

for _p in ("/opt/trn_rl_repo", "/root/.axon_site/_ro/trn_rl_repo"):
    if _p not in sys.path:
        sys.path.insert(0, _p)

import numpy as np

B, S, H, NH, HS = 4, 1024, 1024, 16, 64
P = 128
QR = 512          # query rows per core
EPS = 1e-12
N_CORES = 8

# row offsets inside the f32 aux bundle
R_XRES = 0            # [512, 1024]
R_QB = 512            # [1, 1024]
R_KB = 513
R_MASK = 514
R_VB = 515
R_OB = 516
R_GAMMA = 517
R_BETA = 518
AUX_ROWS = 519

_CACHE = {}


def _build_nc():
    import concourse.mybir as mybir
    import concourse.tile as tile
    from concourse import bacc

    f32 = mybir.dt.float32
    bf16 = mybir.dt.bfloat16
    f8 = mybir.dt.float8e4
    DR = mybir.MatmulPerfMode.DoubleRow
    Alu = mybir.AluOpType
    Act = mybir.ActivationFunctionType

    nc = bacc.Bacc("TRN2", target_bir_lowering=False)

    # ---- packed per-core DRAM I/O ----
    # wx rows: [0:1024] x^T, [1024:2048] qw^T, [2048:3072] kw^T,
    #          [3072:4096] vw^T, [4096:5120] ow^T   (all [in, out] layout)
    wx_d = nc.declare_dram_parameter("wx", [5 * H, S], f8, isOutput=False)
    aux_d = nc.declare_dram_parameter("aux", [AUX_ROWS, H], f32, isOutput=False)
    out_d = nc.declare_dram_parameter("out", [QR, H], f32, isOutput=True)

    xt_d = wx_d[0:H, :]
    qwt_d = wx_d[H : 2 * H, :]
    kwt_d = wx_d[2 * H : 3 * H, :]
    vwt_d = wx_d[3 * H : 4 * H, :]
    owt_d = wx_d[4 * H : 5 * H, :]
    xres_d = aux_d[R_XRES:QR, :]

    KT8 = H // P   # 8 tiles along any 1024 dim

    with tile.TileContext(nc) as tc:
        with (
            tc.tile_pool(name="consts", bufs=1) as consts,
            tc.tile_pool(name="bigs", bufs=1) as bigs,
            tc.tile_pool(name="pers", bufs=2) as pers,
            tc.tile_pool(name="wbuf", bufs=4) as wbuf,
            tc.tile_pool(name="exps", bufs=3) as exps,
            tc.tile_pool(name="small", bufs=4) as small,
            tc.tile_pool(name="resid", bufs=2) as resid,
            tc.tile_pool(name="onorm", bufs=2) as onorm_pool,
            tc.tile_pool(name="norm", bufs=2) as norm_pool,
            tc.tile_pool(name="mm_ps", bufs=2, space="PSUM") as mm_ps,
            tc.tile_pool(name="sc_ps", bufs=2, space="PSUM") as sc_ps,
            tc.tile_pool(name="ctx_ps", bufs=1, space="PSUM") as ctx_ps,
            tc.tile_pool(name="dram", bufs=2, space="DRAM") as dram_pool,
        ):
            # ---- constants / small inputs (from aux rows) ----
            qkm_sb = consts.tile([P, 3, KT8], f32)
            nc.sync.dma_start(
                out=qkm_sb,
                in_=aux_d[R_QB : R_QB + 3, :].rearrange("r (io p) -> p r io", p=P),
            )
            qb_sb = qkm_sb[:, 0, :]
            kb_sb = qkm_sb[:, 1, :]
            mask_sb = qkm_sb[:, 2, :]
            vbb_sb = consts.tile([P, H], f32)
            nc.sync.dma_start(
                out=vbb_sb, in_=aux_d[R_VB : R_VB + 1, :].to_broadcast([P, H])
            )
            obb_sb = consts.tile([P, H], f32)
            nc.sync.dma_start(
                out=obb_sb, in_=aux_d[R_OB : R_OB + 1, :].to_broadcast([P, H])
            )
            gammab_sb = consts.tile([P, H], f32)
            nc.sync.dma_start(
                out=gammab_sb,
                in_=aux_d[R_GAMMA : R_GAMMA + 1, :].to_broadcast([P, H]),
            )
            betab_sb = consts.tile([P, H], f32)
            nc.sync.dma_start(
                out=betab_sb, in_=aux_d[R_BETA : R_BETA + 1, :].to_broadcast([P, H])
            )
            eps_sb = consts.tile([P, 1], f32)
            nc.vector.memset(eps_sb, EPS)
            ones64 = consts.tile([1, HS], f32)
            nc.vector.memset(ones64, 1.0)
            # warm the ACT exp table during the load phase so the first real
            # exp doesn't pay the ~2.7us ACT_TABLE_LOAD
            actwarm = consts.tile([P, 1], f32)
            nc.scalar.activation(out=actwarm, in_=eps_sb, func=Act.Exp)

            # ---- x^T in SBUF: [128(i), 8(i_outer), 1024(s)] bf16 ----
            xt_sb = pers.tile([P, KT8, S], f8, tag="xt")
            xt_r = xt_d.rearrange("(io p) s -> p io s", p=P)
            nc.sync.dma_start(out=xt_sb, in_=xt_r)

            # ---- persistent big tensors ----
            QT = pers.tile([P, KT8, QR], bf16, tag="QT")   # Q^T  [o, q]
            KT = bigs.tile([P, KT8, S], bf16)         # K^T  [o, s]
            Vaug = bigs.tile([P, KT8, NH, HS + 1], f8)  # V   [s, h, d(+ones)]
            ctxT = bigs.tile([P, KT8, QR], f8)        # ctx^T [j, q]

            nc.vector.memset(Vaug[:, :, :, HS : HS + 1], 1.0)

            def load_wt(dram, eng):
                w = wbuf.tile([P, KT8, H], f8, tag="wt")
                w_r = dram.rearrange("(io p) o -> p io o", p=P)
                eng.dma_start(out=w, in_=w_r)
                return w

            # ---- weight loads on separate DGE queues (concurrent) ----
            qwt = load_wt(qwt_d, nc.scalar)
            kwt = load_wt(kwt_d, nc.gpsimd)
            vwt = load_wt(vwt_d, nc.sync)
            owt = load_wt(owt_d, nc.scalar)

            def emit_v(st, oh):
                ps = mm_ps.tile([P, QR], f32, tag="mm")
                for it in range(KT8 // 2):
                    nc.tensor.matmul(
                        ps,
                        xt_sb[:, 2 * it : 2 * it + 2, st * P : (st + 1) * P],
                        vwt[:, 2 * it : 2 * it + 2, oh * QR : (oh + 1) * QR],
                        start=(it == 0),
                        stop=(it == KT8 // 2 - 1),
                        perf_mode=DR,
                    )
                nc.vector.tensor_tensor(
                    out=Vaug[:, st, oh * (NH // 2) : (oh + 1) * (NH // 2), 0:HS],
                    in0=ps.rearrange("p (h d) -> p h d", d=HS),
                    in1=vbb_sb[:, oh * QR : (oh + 1) * QR].rearrange(
                        "p (h d) -> p h d", d=HS
                    ),
                    op=Alu.add,
                )

            def emit_q(ot):
                ps = mm_ps.tile([P, QR], f32, tag="mm")
                for it in range(KT8 // 2):
                    nc.tensor.matmul(
                        ps,
                        qwt[:, 2 * it : 2 * it + 2, ot * P : (ot + 1) * P],
                        xt_sb[:, 2 * it : 2 * it + 2, 0:QR],
                        start=(it == 0),
                        stop=(it == KT8 // 2 - 1),
                        perf_mode=DR,
                    )
                nc.vector.tensor_scalar_add(
                    out=QT[:, ot, :], in0=ps, scalar1=qb_sb[:, ot : ot + 1]
                )

            # prefetch residual rows and fold in the output bias while the
            # attention stage runs (Pool engine is idle then)
            xres_r = xres_d.rearrange("(st p) m -> p st m", p=P)
            xo = bigs.tile([P, QR // P, H], f32)
            for st in range(QR // P):
                xr = resid.tile([P, H], f32, tag="xr")
                nc.sync.dma_start(out=xr, in_=xres_r[:, st, :])
                nc.gpsimd.tensor_tensor(
                    out=xo[:, st, :], in0=xr, in1=obb_sb, op=Alu.add
                )

            # ---- attention ----
            def emit_kt(ot):
                for sh in range(2):
                    kps = mm_ps.tile([P, QR], f32, tag="mm", name="kps")
                    for it in range(KT8 // 2):
                        nc.tensor.matmul(
                            kps,
                            kwt[:, 2 * it : 2 * it + 2, ot * P : (ot + 1) * P],
                            xt_sb[:, 2 * it : 2 * it + 2, sh * QR : (sh + 1) * QR],
                            start=(it == 0),
                            stop=(it == KT8 // 2 - 1),
                            perf_mode=DR,
                        )
                    nc.vector.tensor_scalar_add(
                        out=KT[:, ot, sh * QR : (sh + 1) * QR],
                        in0=kps,
                        scalar1=kb_sb[:, ot : ot + 1],
                    )

            emit_q(0)
            emit_kt(0)
            emit_q(1)

            # V groups interleave into the attention stream just in time:
            # pair 0 consumes oh=0 s-tiles as its ctx matmuls need them,
            # pairs 1..3 cover the oh=1 half (needed from pair 4 on)
            v1_sched = {1: [0, 1, 2], 2: [3, 4, 5], 3: [6, 7]}

            for oo in range(NH // 2):
                heads = (2 * oo, 2 * oo + 1)  # partition offsets 0, 64

                expS = exps.tile([P, KT8, 2, QR], f8, tag="expS", name="expS")
                cpss = [
                    ctx_ps.tile([P, QR], f32, tag=f"ctx{j}", name=f"ctx{j}")
                    for j in range(2)
                ]

                def emit_scores(so):
                    sps = sc_ps.tile([P, 2 * QR], f32, tag="sc", name="sps")
                    for j in range(2):
                        po = j * HS
                        nc.tensor.matmul(
                            sps[:, j * QR : (j + 1) * QR],
                            KT[po : po + HS, oo, so * P : (so + 1) * P],
                            QT[po : po + HS, oo, :],
                            start=True,
                            stop=True,
                        )
                    nc.scalar.activation(
                        out=expS[:, so, :, :],
                        in_=sps.rearrange("p (j q) -> p j q", q=QR),
                        func=Act.Exp,
                        bias=mask_sb[:, so : so + 1],
                        scale=0.125,
                    )

                def emit_ctx_pair(t):
                    for j, h in enumerate(heads):
                        nc.tensor.matmul(
                            cpss[j][0 : HS + 1, :],
                            Vaug[:, 2 * t : 2 * t + 2, h, :],
                            expS[:, 2 * t : 2 * t + 2, j, :],
                            start=(t == 0),
                            stop=(t == KT8 // 2 - 1),
                            perf_mode=DR,
                        )

                vq = list(v1_sched.get(oo, []))
                emit_scores(0)
                emit_scores(1)
                if oo == 0:
                    emit_v(0, 0)
                    emit_v(1, 0)
                emit_ctx_pair(0)
                if oo + 1 < NH // 2:
                    emit_kt(oo + 1)
                if oo + 2 < NH // 2:
                    emit_q(oo + 2)
                for t in range(1, KT8 // 2):
                    emit_scores(2 * t)
                    emit_scores(2 * t + 1)
                    if oo == 0:
                        emit_v(2 * t, 0)
                        emit_v(2 * t + 1, 0)
                    elif vq:
                        emit_v(vq.pop(0), 1)
                    emit_ctx_pair(t)
                while vq:
                    emit_v(vq.pop(0), 1)
                # normalize by softmax denominator (row HS of cps).
                # Both heads' psums are copied into one SBUF tile with two
                # cheap DVE ops (banks free immediately); one reciprocal and
                # one broadcast DMA cover the pair, and the multiplies run
                # on the idle Pool engine off the critical path.  The LAST
                # pair sits on the tail critical path, so it skips the DRAM
                # round-trip: PE broadcasts 1/denom into the freed scores
                # PSUM and DVE multiplies directly.
                last = oo == NH // 2 - 1
                cc = norm_pool.tile([HS + 1, 2, QR], f32, tag="cc")
                for j in range(2):
                    nc.vector.tensor_copy(
                        out=cc[:, j, :], in_=cpss[j][0 : HS + 1, :]
                    )
                if last:
                    rdl = norm_pool.tile([1, 2, QR], f32, tag="rd")
                    nc.vector.reciprocal(out=rdl, in_=cc[HS : HS + 1, :, :])
                    bps = sc_ps.tile([P, 2 * QR], f32, tag="sc", name="bps")
                    for j in range(2):
                        nc.tensor.matmul(
                            bps[0:HS, j * QR : (j + 1) * QR],
                            ones64,
                            rdl[:, j, :],
                            start=True,
                            stop=True,
                        )
                    nc.vector.tensor_tensor(
                        out=ctxT[0:HS, oo, :],
                        in0=cc[0:HS, 0, :],
                        in1=bps[0:HS, 0:QR],
                        op=Alu.mult,
                    )
                    stage = small.tile([HS, QR], f8, tag="cstage")
                    nc.vector.tensor_tensor(
                        out=stage, in0=cc[0:HS, 1, :], in1=bps[0:HS, QR : 2 * QR],
                        op=Alu.mult,
                    )
                    nc.gpsimd.dma_start(out=ctxT[HS : 2 * HS, oo, :], in_=stage)
                else:
                    rd = norm_pool.tile([1, 2, QR], f32, tag="rd")
                    nc.vector.reciprocal(out=rd, in_=cc[HS : HS + 1, :, :])
                    rdd = dram_pool.tile([1, 2 * QR], f32, tag="rdd")
                    nc.gpsimd.dma_start(
                        out=rdd, in_=rd.rearrange("o j q -> o (j q)")
                    )
                    rdb = norm_pool.tile([P, 2, QR], f32, tag="rdb")
                    nc.gpsimd.dma_start(
                        out=rdb[0:HS, :, :],
                        in_=rdd[:, :].rearrange(
                            "o (j q) -> o j q", q=QR
                        ).to_broadcast([HS, 2, QR]),
                    )
                    nc.gpsimd.tensor_tensor(
                        out=ctxT[0:HS, oo, :],
                        in0=cc[0:HS, 0, :],
                        in1=rdb[0:HS, 0, :],
                        op=Alu.mult,
                    )
                    stage = small.tile([HS, QR], f8, tag="cstage")
                    nc.gpsimd.tensor_tensor(
                        out=stage, in0=cc[0:HS, 1, :], in1=rdb[0:HS, 1, :],
                        op=Alu.mult,
                    )
                    nc.gpsimd.dma_start(out=ctxT[HS : 2 * HS, oo, :], in_=stage)

            # ---- y = ctx @ ow^T + (ob + residual) ----
            # drain each psum half with a single add against the prefetched
            # (residual + output bias) rows, then layernorm on the result
            ysb = bigs.tile([P, QR // P, H], f32)
            for st in range(QR // P):
                for oh in range(2):
                    ps = mm_ps.tile([P, QR], f32, tag="mm")
                    for jo in range(KT8 // 2):
                        nc.tensor.matmul(
                            ps,
                            ctxT[:, 2 * jo : 2 * jo + 2, st * P : (st + 1) * P],
                            owt[:, 2 * jo : 2 * jo + 2, oh * QR : (oh + 1) * QR],
                            start=(jo == 0),
                            stop=(jo == KT8 // 2 - 1),
                            perf_mode=DR,
                        )
                    nc.vector.tensor_tensor(
                        out=ysb[:, st, oh * QR : (oh + 1) * QR],
                        in0=ps,
                        in1=xo[:, st, oh * QR : (oh + 1) * QR],
                        op=Alu.add,
                    )

                # ---- layernorm over the 1024 free dim ----
                yr = ysb[:, st, :].rearrange("p (g d) -> p g d", d=QR)
                stats = small.tile([P, 2, 6], f32, tag="stats")
                for g in range(2):
                    nc.vector.bn_stats(out=stats[:, g, :], in_=yr[:, g, :])
                mv = small.tile([P, 2], f32, tag="mv")
                nc.vector.bn_aggr(out=mv, in_=stats)
                rstd = small.tile([P, 1], f32, tag="rstd")
                nc.scalar.activation(
                    out=rstd, in_=mv[:, 1:2], func=Act.Sqrt, bias=eps_sb, scale=1.0
                )
                nc.vector.reciprocal(out=rstd, in_=rstd)
                on = onorm_pool.tile([P, H], f32, tag="on")
                nc.vector.tensor_scalar(
                    out=on,
                    in0=ysb[:, st, :],
                    scalar1=mv[:, 0:1],
                    scalar2=rstd,
                    op0=Alu.subtract,
                    op1=Alu.mult,
                )
                nc.gpsimd.tensor_tensor(out=on, in0=on, in1=gammab_sb, op=Alu.mult)
                nc.gpsimd.tensor_tensor(out=on, in0=on, in1=betab_sb, op=Alu.add)
                nc.scalar.dma_start(
                    out=out_d.rearrange("(st p) m -> p st m", p=P)[:, st, :], in_=on
                )

    nc.compile()
    return nc


def _get_nc():
    if "nc" not in _CACHE:
        _CACHE["nc"] = _build_nc()
    return _CACHE["nc"]


def _make_in_maps(inputs):
    import ml_dtypes

    f8 = ml_dtypes.float8_e4m3
    hs = np.asarray(inputs["hidden_states"], dtype=np.float32).reshape(B, S, H)
    am = np.asarray(inputs["attention_mask"], dtype=np.float32).reshape(B, S)

    # shared fp8 weight block [4096, 1024]: qw^T, kw^T, vw^T, ow^T
    wblk = np.empty((4 * H, H), dtype=f8)
    for i, nm in enumerate(("qw", "kw", "vw", "ow")):
        wblk[i * H : (i + 1) * H] = np.asarray(inputs[nm], np.float32).T.astype(f8)

    # shared aux rows (everything but xres + mask)
    aux_shared = np.zeros((AUX_ROWS - QR, H), dtype=np.float32)
    aux_shared[R_QB - QR] = np.asarray(inputs["qb"], np.float32)
    aux_shared[R_KB - QR] = np.asarray(inputs["kb"], np.float32)
    aux_shared[R_VB - QR] = np.asarray(inputs["vb"], np.float32)
    aux_shared[R_OB - QR] = np.asarray(inputs["ob"], np.float32)
    aux_shared[R_GAMMA - QR] = np.asarray(inputs["gamma"], np.float32)
    aux_shared[R_BETA - QR] = np.asarray(inputs["beta"], np.float32)

    in_maps = []
    for c in range(N_CORES):
        b, half = divmod(c, 2)
        x = hs[b]
        m = am[b]
        if half:
            x = np.roll(x, -QR, axis=0)
            m = np.roll(m, -QR)
        wx = np.empty((5 * H, S), dtype=f8)
        wx[0:H] = x.T.astype(f8)
        wx[H:] = wblk
        aux = np.empty((AUX_ROWS, H), dtype=np.float32)
        aux[0:QR] = x[:QR]
        aux[QR:] = aux_shared
        aux[R_MASK] = m
        in_maps.append({"wx": wx, "aux": aux})
    return in_maps


def _gather(results):
    out = np.empty((B, S, H), dtype=np.float32)
    for c in range(N_CORES):
        b, half = divmod(c, 2)
        out[b, half * QR : (half + 1) * QR, :] = results[c]["out"]
    return out


def run_on_hw(inputs, **kwargs):
    """Run on the 8 NeuronCores; returns (full_output, BassKernelResults)."""
    from concourse import bass_utils

    nc = _get_nc()
    in_maps = _make_in_maps(inputs)
    res = bass_utils.run_bass_kernel_spmd(
        nc, in_maps, core_ids=list(range(N_CORES)), **kwargs
    )
    return _gather(res.results), res


def kernel(**inputs) -> np.ndarray:
    out, _ = run_on_hw(inputs)
    return out
